# revision 15
# baseline (speedup 1.0000x reference)
"""AGFAttention Trainium2 kernel — 8-core SPMD, head-sharded.

Sharding: core c owns heads {2c, 2c+1}, both batches -> 4 (b,h) pairs/core.
All attention work is local; the out-projection is row-parallel: each core
emits a full-shape fp32 partial, host sums partials and adds bout.

Numerics: scores must match the fp32 reference to ~1e-6, else top-64
membership flips (score gaps ~7e-3) cost ~3% error per flipped row. So the
QKV and score matmuls and the top-k peel run in fp32; everything downstream
of the masked exp (U, v', res, Wout) is fp16.

Per (b,h) pair (N=2048):
  A. S row-major = Q K^T (fp32) -> fp32 SBUF; exact top-64 value t per row
     via 8 rounds of DVE max8 + match_replace peel.
  B. S^T - t in one fp32 matmul pass (65-row extended operands: K^T gets a
     ones row, Q^T gets a -t row); U^T = exp(psum)*(psum>=0) -> fp16.
  C. w_{i+1} = U v'_i, ones-column gives row-sums d; v'_{i+1} = w/d (fp16).
     res^T accumulated in PSUM by alpha*I matmuls (transpose+scale+acc);
     rp banks pre-zeroed by dummy matmuls so interleaved chunk groups can
     all accumulate with start=False (a start=True would clear the whole
     bank's has_written bits, wiping sibling chunks' accumulation state).
  D. partial_out = res^T.T @ Wout_rows(core) in fp16.
"""
import numpy as np
from contextlib import ExitStack

import concourse.bass as bass
import concourse.tile as tile
from concourse import bacc, mybir
from concourse.bass_utils import run_bass_kernel_spmd

F16 = mybir.dt.float16
F32 = mybir.dt.float32

B, N, DIM = 2, 2048, 1024
H, DH, ORDER = 16, 64, 3
SCALE = DH ** -0.5
HPC = H // 8            # heads per core = 2
TOK = B * N             # 4096
NT = N // 128           # 16 row tiles per (b,h)
MC = 512                # matmul free chunk
NEG = -1e30


def build_nc():
    nc = bacc.Bacc("TRN2", target_bir_lowering=False, debug=False, num_devices=8)

    # w cols: [q0*SCALE|k0 (128)] [q1*SCALE|k1 (128)] [v0|v1 (128)]
    xt_d = nc.dram_tensor("xt", [DIM + 1, TOK], F32, kind="ExternalInput").ap()
    w_d = nc.dram_tensor("w", [DIM + 1, 384], F32, kind="ExternalInput").ap()
    wout_d = nc.dram_tensor("wout", [128, DIM], F16, kind="ExternalInput").ap()
    aI_d = nc.dram_tensor("aI", [2 * HPC * ORDER, 128, 128], F16,
                          kind="ExternalInput").ap()
    ones_d = nc.dram_tensor("ones", [1, TOK], F32, kind="ExternalInput").ap()
    eye_d = nc.dram_tensor("eye64", [64, 64], F16, kind="ExternalInput").ap()
    out_d = nc.dram_tensor("out", [TOK, DIM], F32, kind="ExternalOutput").ap()
    import os
    DBG = bool(int(os.environ.get("BASSDEBUG", "0")))
    if DBG:
        dbg_ut = nc.dram_tensor("dbg_ut", [128, N], F16, kind="ExternalOutput").ap()
        dbg_m = nc.dram_tensor("dbg_m", [128, N], F16, kind="ExternalOutput").ap()
        dbg_nt = nc.dram_tensor("dbg_nt", [128, NT], F32, kind="ExternalOutput").ap()

    with tile.TileContext(nc) as tc, ExitStack() as ctx:
        wpool = ctx.enter_context(tc.tile_pool(name="wpool", bufs=1))
        xpool = ctx.enter_context(tc.tile_pool(name="xpool", bufs=3))
        qk = ctx.enter_context(tc.tile_pool(name="qk", bufs=1))
        peel = ctx.enter_context(tc.tile_pool(name="peel", bufs=2))
        mk = ctx.enter_context(tc.tile_pool(name="mk", bufs=1))
        tvp = ctx.enter_context(tc.tile_pool(name="tvp", bufs=3))
        ut = ctx.enter_context(tc.tile_pool(name="ut", bufs=16))
        vb = ctx.enter_context(tc.tile_pool(name="vb", bufs=3))
        small = ctx.enter_context(tc.tile_pool(name="small", bufs=4))
        rest = ctx.enter_context(tc.tile_pool(name="rest", bufs=1))
        dram = ctx.enter_context(tc.tile_pool(name="dram", bufs=2, space="DRAM"))
        # PSUM (8 banks): ps_s 2x[128,1024]=4, ps_w 2x1=2, ps_o 2x1=2
        ps_s = ctx.enter_context(tc.tile_pool(name="ps_s", bufs=2, space="PSUM"))
        ps_w = ctx.enter_context(tc.tile_pool(name="ps_w", bufs=2, space="PSUM"))
        ps_o = ctx.enter_context(tc.tile_pool(name="ps_o", bufs=2, space="PSUM"))

        # ---------------- constants / weights ----------------
        wt = []
        for kt in range(8):
            t = wpool.tile([128, 384], F32, tag=f"w{kt}")
            nc.sync.dma_start(t[:], w_d[kt * 128:(kt + 1) * 128, :])
            wt.append(t)
        wlast = wpool.tile([1, 384], F32, tag="wlast")
        nc.sync.dma_start(wlast[:], w_d[DIM:DIM + 1, :])
        woutt = wpool.tile([128, DIM], F16, tag="wout")
        nc.sync.dma_start(woutt[:], wout_d[:])
        aIt = wpool.tile([128, 2 * HPC * ORDER * 128], F16, tag="aI")
        nc.sync.dma_start(aIt[:].rearrange("p (s f) -> p s f", f=128),
                          aI_d[:].rearrange("s p f -> p s f"))
        eye = wpool.tile([64, 64], F16, tag="eye")
        nc.sync.dma_start(eye[:], eye_d[:])
        zeros = wpool.tile([64, MC], F16, tag="zeros")
        nc.vector.memset(zeros[:], 0.0)

        resT = rest.tile([128, TOK], F16, tag="resT")
        Tvs = {}

        for h in range(HPC):
            # ---------------- QKV for this head ----------------
            tqh = qk.tile([65, TOK], F32, tag="tq")
            tkh = qk.tile([65, TOK], F32, tag="tk")
            nc.sync.dma_start(tkh[64:65, :], ones_d[:])
            mts = [h]                      # M-tile: q_h|k_h cols
            if h == 0:
                tv0t = qk.tile([64, TOK], F16, tag="tv0")
                tv1t = qk.tile([64, TOK], F16, tag="tv1")
                Tvs[0], Tvs[1] = tv0t, tv1t
                mts.append(2)              # v0|v1 cols
            for ch in range(TOK // MC):
                xs = []
                for kt in range(9):
                    rows = 128 if kt < 8 else 1
                    xtile = xpool.tile([rows, MC], F32, tag=f"x{kt % 3}")
                    nc.sync.dma_start(
                        xtile[:rows, :], xt_d[kt * 128:kt * 128 + rows,
                                              ch * MC:(ch + 1) * MC])
                    xs.append(xtile)
                for mt in mts:
                    ps = ps_o.tile([128, MC], F32, tag="o")
                    for kt in range(8):
                        nc.tensor.matmul(
                            ps[:], wt[kt][:, mt * 128:(mt + 1) * 128],
                            xs[kt][:], start=(kt == 0), stop=False)
                    nc.tensor.matmul(
                        ps[:], wlast[:1, mt * 128:(mt + 1) * 128],
                        xs[8][:1, :], start=False, stop=True)
                    if mt == 2:
                        nc.scalar.copy(Tvs[0][0:64, ch * MC:(ch + 1) * MC],
                                       ps[0:64, :])
                        nc.scalar.copy(Tvs[1][0:64, ch * MC:(ch + 1) * MC],
                                       ps[64:128, :])
                    else:
                        nc.scalar.copy(tqh[0:DH, ch * MC:(ch + 1) * MC],
                                       ps[0:64, :])
                        nc.scalar.copy(tkh[0:DH, ch * MC:(ch + 1) * MC],
                                       ps[64:128, :])

            for b in range(2):
                t0 = b * N
                ntbuf = small.tile([128, NT], F32, tag="ntbuf")

                # ---- A: S row-major + fp32 peel ----
                for j in range(NT):
                    s32 = peel.tile([128, N], F32, tag="peelA")
                    for half in range(2):
                        ps = ps_s.tile([128, 1024], F32, tag="s")
                        for mc in range(2):
                            o = half * 1024 + mc * MC
                            nc.tensor.matmul(
                                ps[:, mc * MC:(mc + 1) * MC],
                                tqh[0:DH, t0 + j * 128:t0 + (j + 1) * 128],
                                tkh[0:DH, t0 + o:t0 + o + MC],
                                start=True, stop=True)
                        nc.scalar.copy(s32[:, half * 1024:(half + 1) * 1024],
                                       ps[:])
                    s32b = peel.tile([128, N], F32, tag="peelB")
                    cur, nxt = s32, s32b
                    tv = None
                    for r in range(8):
                        tv = tvp.tile([128, 8], F32, tag="tv")
                        nc.vector.max(tv[:], cur[:])
                        if r < 7:
                            nc.vector.match_replace(nxt[:], tv[:], cur[:], NEG)
                            cur, nxt = nxt, cur
                    nc.scalar.mul(ntbuf[:, j:j + 1], tv[:, 7:8], -1.0)

                if DBG and h == 0 and b == 0:
                    nc.sync.dma_start(dbg_nt[:], ntbuf[:])
                tb = dram.tile([N], F32, tag="tb")
                nc.sync.dma_start(tb[:].rearrange("(j p) -> p j", p=128),
                                  ntbuf[:])
                nc.sync.dma_start(tqh[64:65, t0:t0 + N],
                                  tb[:].rearrange("(o n) -> o n", o=1))

                # ---- B: S^T - t -> U^T (fp16) ----
                uts = []
                for i in range(NT):
                    e = ut.tile([128, N], F16, tag="ut")
                    m = mk.tile([128, N], F16, tag="mk")
                    for half in range(2):
                        ps = ps_s.tile([128, 1024], F32, tag="s")
                        for nch in range(2):
                            o = half * 1024 + nch * MC
                            nc.tensor.matmul(
                                ps[:, nch * MC:(nch + 1) * MC],
                                tkh[:, t0 + i * 128:t0 + (i + 1) * 128],
                                tqh[:, t0 + o:t0 + o + MC],
                                start=True, stop=True)
                        nc.scalar.activation(
                            e[:, half * 1024:(half + 1) * 1024], ps[:],
                            mybir.ActivationFunctionType.Exp)
                        nc.vector.tensor_scalar(
                            m[:, half * 1024:(half + 1) * 1024], ps[:],
                            -1e-5, None, op0=mybir.AluOpType.is_ge)
                    if DBG and h == 0 and b == 0 and i == 0:
                        nc.sync.dma_start(dbg_m[:], m[:])
                    nc.vector.tensor_tensor(e[:], e[:], m[:],
                                            op=mybir.AluOpType.mult)
                    if DBG and h == 0 and b == 0 and i == 0:
                        nc.sync.dma_start(dbg_ut[:], e[:])
                    uts.append(e)

                # ---- C: recursion ----
                vcur = vb.tile([128, NT * 65], F16, tag="vb")
                for j in range(NT):
                    pst = ps_w.tile([128, 65], F32, tag="wps")
                    nc.tensor.matmul(
                        pst[:, 0:64],
                        Tvs[h][0:DH, t0 + j * 128:t0 + (j + 1) * 128],
                        eye[:], start=True, stop=True)
                    nc.scalar.copy(vcur[:, j * 65:j * 65 + 64], pst[:, 0:64])
                nc.vector.memset(
                    vcur[:].rearrange("p (j o) -> p j o", o=65)[:, :, 64:65],
                    1.0)

                pair = h * 2 + b
                rp = []
                for _q in range(2):
                    rpt = ps_s.tile([64, 1024], F32, tag="s")
                    rp.append(rpt)
                    for bk in range(2):
                        nc.tensor.matmul(rpt[:, bk * MC:(bk + 1) * MC],
                                         eye[:], zeros[:],
                                         start=True, stop=False)
                for step in range(ORDER):
                    vnxt = vb.tile([128, NT * 65], F16, tag="vb")
                    for j in range(NT):
                        ps = ps_w.tile([128, 65], F32, tag="wps")
                        for i in range(NT):
                            nc.tensor.matmul(
                                ps[:], uts[i][:, j * 128:(j + 1) * 128],
                                vcur[:, i * 65:(i + 1) * 65],
                                start=(i == 0), stop=(i == NT - 1))
                        rd = small.tile([128, 1], F32, tag="rd")
                        nc.vector.reciprocal(rd[:], ps[:, 64:65])
                        nc.vector.tensor_scalar(
                            vnxt[:, j * 65:(j + 1) * 65], ps[:], rd[:], None,
                            op0=mybir.AluOpType.mult)
                        aslot = pair * ORDER + step
                        nc.tensor.matmul(
                            rp[j // 8][:, (j % 8) * 128:(j % 8 + 1) * 128],
                            vnxt[:, j * 65:j * 65 + 64],
                            aIt[:, aslot * 128:(aslot + 1) * 128],
                            start=False, stop=(step == ORDER - 1))
                    vcur = vnxt

                for q in range(2):
                    nc.scalar.copy(
                        resT[h * DH:(h + 1) * DH,
                             t0 + q * 1024:t0 + (q + 1) * 1024], rp[q][:])

        # ---------------- out projection (partial) ----------------
        for tt in range(TOK // 128):
            for oc in range(DIM // MC):
                ps = ps_o.tile([128, MC], F32, tag="o")
                nc.tensor.matmul(
                    ps[:], resT[:, tt * 128:(tt + 1) * 128],
                    woutt[:, oc * MC:(oc + 1) * MC], start=True, stop=True)
                ot = xpool.tile([128, MC], F32, tag="ot")
                nc.scalar.copy(ot[:], ps[:])
                nc.sync.dma_start(
                    out_d[tt * 128:(tt + 1) * 128, oc * MC:(oc + 1) * MC],
                    ot[:])

    nc.compile()
    return nc


_NC = None
_LAST_IN_MAPS = None


def make_in_maps(x, Wqkv, bqkv, Wout, bout, alphas_raw):
    x = np.asarray(x, np.float32)
    Wqkv = np.asarray(Wqkv, np.float32)
    bqkv = np.asarray(bqkv, np.float32)
    Wout = np.asarray(Wout, np.float32)
    alphas = 1.0 / (1.0 + np.exp(-np.asarray(alphas_raw, np.float32)))

    xt = np.concatenate([x.reshape(TOK, DIM).T,
                         np.ones((1, TOK), np.float32)], 0).astype(np.float32)
    ones = np.ones((1, TOK), np.float32)
    eye128 = np.eye(128, dtype=np.float32)
    eye64 = np.eye(64, dtype=np.float16)

    inner = H * DH
    wb = np.concatenate([Wqkv, bqkv[None, :]], 0)   # [1025, 3072]
    in_maps = []
    for c in range(8):
        hs = [2 * c, 2 * c + 1]
        cols = []
        for h in hs:                       # q_h | k_h per head
            cols.append(np.arange(h * DH, (h + 1) * DH))
            cols.append(np.arange(inner + h * DH, inner + (h + 1) * DH))
        for h in hs:                       # v0 | v1
            cols.append(np.arange(2 * inner + h * DH,
                                  2 * inner + (h + 1) * DH))
        cols = np.concatenate(cols)
        wc = wb[:, cols].copy()
        wc[:, 0:DH] *= SCALE               # q0
        wc[:, 128:128 + DH] *= SCALE       # q1
        rows = np.concatenate([np.arange(h * DH, (h + 1) * DH) for h in hs])
        woc = Wout[rows, :]
        aI = np.zeros((2 * HPC * ORDER, 128, 128), np.float32)
        for pair in range(2 * HPC):
            h = pair // 2                  # pair = h*2 + b
            for step in range(ORDER):
                aI[pair * ORDER + step] = alphas[step + 1, hs[h]] * eye128
        in_maps.append({
            "xt": xt,
            "w": wc.astype(np.float32),
            "wout": woc.astype(np.float16),
            "aI": aI.astype(np.float16),
            "ones": ones,
            "eye64": eye64,
        })
    return in_maps


def kernel(x, Wqkv, bqkv, Wout, bout, alphas_raw):
    global _NC, _LAST_IN_MAPS
    bout = np.asarray(bout, np.float32)
    in_maps = make_in_maps(x, Wqkv, bqkv, Wout, bout, alphas_raw)
    _LAST_IN_MAPS = in_maps
    if _NC is None:
        _NC = build_nc()
    res = run_bass_kernel_spmd(_NC, in_maps, core_ids=list(range(8)))
    out = np.zeros((TOK, DIM), np.float64)
    for c in range(8):
        out += res.results[c]["out"].astype(np.float64)
    out = (out + bout[None, :]).astype(np.float32)
    return out.reshape(B, N, DIM)


# revision 28
# speedup vs baseline: 1.0551x; 1.0551x over previous
"""AGFAttention Trainium2 kernel — 8-core SPMD, head-sharded.

Sharding: core c owns heads {2c, 2c+1}, both batches -> 4 (b,h) pairs/core.
All attention work is local; the out-projection is row-parallel: each core
emits a full-shape fp32 partial, host sums partials and adds bout.

Numerics: scores must match the fp32 reference to ~1e-6, else top-64
membership flips (score gaps ~7e-3) cost ~3% error per flipped row. So the
QKV and score matmuls and the top-k peel run in fp32; everything downstream
of the masked exp (U, v', res, Wout) is fp16.

Per (b,h) pair (N=2048):
  A. S row-major = Q K^T (fp32) -> fp32 SBUF; exact top-64 value t per row
     via 8 rounds of DVE max8 + match_replace peel.
  B. S^T - t in one fp32 matmul pass (65-row extended operands: K^T gets a
     ones row, Q^T gets a -t row); U^T = exp(psum)*(psum>=0) -> fp16.
  C. w_{i+1} = U v'_i, ones-column gives row-sums d; v'_{i+1} = w/d (fp16).
     res^T accumulated in PSUM by alpha*I matmuls (transpose+scale+acc);
     rp banks pre-zeroed by dummy matmuls so interleaved chunk groups can
     all accumulate with start=False (a start=True would clear the whole
     bank's has_written bits, wiping sibling chunks' accumulation state).
  D. partial_out = res^T.T @ Wout_rows(core) in fp16.
"""
import numpy as np
from contextlib import ExitStack

import concourse.bass as bass
import concourse.tile as tile
from concourse import bacc, mybir
from concourse.bass_utils import run_bass_kernel_spmd

F16 = mybir.dt.float16
F32 = mybir.dt.float32

B, N, DIM = 2, 2048, 1024
H, DH, ORDER = 16, 64, 3
SCALE = DH ** -0.5
HPC = H // 8            # heads per core = 2
TOK = B * N             # 4096
NT = N // 128           # 16 row tiles per (b,h)
MC = 512                # matmul free chunk
NEG = -1e30


def build_nc():
    nc = bacc.Bacc("TRN2", target_bir_lowering=False, debug=False, num_devices=8)

    # w cols: [q0*SCALE|k0 (128)] [q1*SCALE|k1 (128)] [v0|v1 (128)]
    xt_d = nc.dram_tensor("xt", [DIM + 1, TOK], F32, kind="ExternalInput").ap()
    w_d = nc.dram_tensor("w", [DIM + 1, 384], F32, kind="ExternalInput").ap()
    wout_d = nc.dram_tensor("wout", [128, DIM], F16, kind="ExternalInput").ap()
    aI_d = nc.dram_tensor("aI", [2 * HPC * ORDER, 128, 128], F16,
                          kind="ExternalInput").ap()
    ones_d = nc.dram_tensor("ones", [1, TOK], F32, kind="ExternalInput").ap()
    eye_d = nc.dram_tensor("eye64", [64, 64], F16, kind="ExternalInput").ap()
    out_d = nc.dram_tensor("out", [TOK, DIM], F32, kind="ExternalOutput").ap()
    import os
    DBG = bool(int(os.environ.get("BASSDEBUG", "0")))
    if DBG:
        dbg_ut = nc.dram_tensor("dbg_ut", [128, N], F16, kind="ExternalOutput").ap()
        dbg_m = nc.dram_tensor("dbg_m", [128, N], F16, kind="ExternalOutput").ap()
        dbg_nt = nc.dram_tensor("dbg_nt", [128, NT], F32, kind="ExternalOutput").ap()

    with tile.TileContext(nc) as tc, ExitStack() as ctx:
        wpool = ctx.enter_context(tc.tile_pool(name="wpool", bufs=1))
        xpool = ctx.enter_context(tc.tile_pool(name="xpool", bufs=2))
        qk = ctx.enter_context(tc.tile_pool(name="qk", bufs=1))
        peel = ctx.enter_context(tc.tile_pool(name="peel", bufs=2))
        mk = ctx.enter_context(tc.tile_pool(name="mk", bufs=1))
        tvp = ctx.enter_context(tc.tile_pool(name="tvp", bufs=2))
        ut = ctx.enter_context(tc.tile_pool(name="ut", bufs=16))
        vb = ctx.enter_context(tc.tile_pool(name="vb", bufs=4))
        small = ctx.enter_context(tc.tile_pool(name="small", bufs=4))
        rest = ctx.enter_context(tc.tile_pool(name="rest", bufs=1))
        dram = ctx.enter_context(tc.tile_pool(name="dram", bufs=2, space="DRAM"))
        # PSUM (8 banks): ps_s 2x[128,1024]=4, ps_w 2x1=2, ps_o 2x1=2
        ps_a = ctx.enter_context(tc.tile_pool(name="ps_a", bufs=1, space="PSUM"))
        ps_s = ctx.enter_context(tc.tile_pool(name="ps_s", bufs=2, space="PSUM"))
        ps_w = ctx.enter_context(tc.tile_pool(name="ps_w", bufs=1, space="PSUM"))
        ps_o = ctx.enter_context(tc.tile_pool(name="ps_o", bufs=1, space="PSUM"))

        # ---------------- constants / weights ----------------
        wt = []
        for kt in range(8):
            t = wpool.tile([128, 384], F32, tag=f"w{kt}")
            nc.sync.dma_start(t[:], w_d[kt * 128:(kt + 1) * 128, :])
            wt.append(t)
        wlast = wpool.tile([1, 384], F32, tag="wlast")
        nc.sync.dma_start(wlast[:], w_d[DIM:DIM + 1, :])
        woutt = wpool.tile([128, DIM], F16, tag="wout")
        nc.sync.dma_start(woutt[:], wout_d[:])
        aIt = wpool.tile([128, 2 * HPC * ORDER * 128], F16, tag="aI")
        nc.sync.dma_start(aIt[:].rearrange("p (s f) -> p s f", f=128),
                          aI_d[:].rearrange("s p f -> p s f"))
        eye = wpool.tile([64, 64], F16, tag="eye")
        nc.sync.dma_start(eye[:], eye_d[:])
        zeros = wpool.tile([64, MC], F16, tag="zeros")
        nc.vector.memset(zeros[:], 0.0)
        biask = wpool.tile([128, 1], F32, tag="biask")
        nc.vector.memset(biask[:], 1e3)

        resT = rest.tile([128, TOK], F16, tag="resT")
        Tvs = {}

        for h in range(HPC):
            # ---------------- QKV for this head ----------------
            tq0 = qk.tile([65, N], F32, tag="tq0")
            tq1 = qk.tile([65, N], F32, tag="tq1")
            tqb = [tq0, tq1]
            tkh = qk.tile([65, TOK], F32, tag="tk")
            nc.sync.dma_start(tkh[64:65, :], ones_d[:])
            if h == 0:
                tv0t = qk.tile([64, TOK], F16, tag="tv0")
                tv1t = qk.tile([64, TOK], F16, tag="tv1")
                Tvs[0], Tvs[1] = tv0t, tv1t

            def qkv_sweep(sweep):
                for ch in range(TOK // MC):
                    xs = []
                    for kt in range(9):
                        rows = 128 if kt < 8 else 1
                        xtile = xpool.tile([rows, MC], F32, tag=f"x{kt % 3}")
                        nc.sync.dma_start(
                            xtile[:rows, :], xt_d[kt * 128:kt * 128 + rows,
                                                  ch * MC:(ch + 1) * MC])
                        xs.append(xtile)
                    for mt in sweep:
                        ps = ps_o.tile([128, MC], F32, tag="o")
                        for kt in range(8):
                            nc.tensor.matmul(
                                ps[:], wt[kt][:, mt * 128:(mt + 1) * 128],
                                xs[kt][:], start=(kt == 0), stop=False)
                        nc.tensor.matmul(
                            ps[:], wlast[:1, mt * 128:(mt + 1) * 128],
                            xs[8][:1, :], start=False, stop=True)
                        if mt == 2:
                            nc.scalar.copy(Tvs[0][0:64, ch * MC:(ch + 1) * MC],
                                           ps[0:64, :])
                            nc.scalar.copy(Tvs[1][0:64, ch * MC:(ch + 1) * MC],
                                           ps[64:128, :])
                        else:
                            cb, cc = divmod(ch * MC, N)
                            nc.scalar.copy(tqb[cb][0:DH, cc:cc + MC],
                                           ps[0:64, :])
                            nc.scalar.copy(tkh[0:DH, ch * MC:(ch + 1) * MC],
                                           ps[64:128, :])

            qkv_sweep([h])

            def phase_A(b):
                t0 = b * N
                ntbuf = small.tile([128, NT], F32, tag="ntbuf")

                # ---- A: S row-major + fp32 peel ----
                for j in range(NT):
                    s32 = peel.tile([128, N], F32, tag="peelA")
                    for half in range(2):
                        ps = ps_a.tile([128, 1024], F32, tag="a")
                        for mc in range(2):
                            o = half * 1024 + mc * MC
                            nc.tensor.matmul(
                                ps[:, mc * MC:(mc + 1) * MC],
                                tqb[b][0:DH, j * 128:(j + 1) * 128],
                                tkh[0:DH, t0 + o:t0 + o + MC],
                                start=True, stop=True)
                        nc.scalar.copy(s32[:, half * 1024:(half + 1) * 1024],
                                       ps[:])
                    s32b = peel.tile([128, N], F32, tag="peelB")
                    cur, nxt = s32, s32b
                    tv = None
                    for r in range(8):
                        tv = tvp.tile([128, 8], F32, tag="tv")
                        nc.vector.max(tv[:], cur[:])
                        if r < 7:
                            nc.vector.match_replace(nxt[:], tv[:], cur[:], NEG)
                            cur, nxt = nxt, cur
                    nc.scalar.mul(ntbuf[:, j:j + 1], tv[:, 7:8], -1.0)

                for g in range(4):
                    tb = dram.tile([MC], F32, tag="tb")
                    nc.sync.dma_start(
                        tb[:].rearrange("(j p) -> p j", p=128),
                        ntbuf[:, g * 4:(g + 1) * 4])
                    nc.sync.dma_start(
                        tqb[b][64:65, g * MC:(g + 1) * MC],
                        tb[:].rearrange("(o n) -> o n", o=1))

            def phase_BC(b, mult_on_dve=False):
                t0 = b * N
                # ---- B: S^T - t -> U^T (fp16) ----
                uts = []
                for i in range(NT):
                    e = ut.tile([128, N], F16, tag="ut")
                    m = mk.tile([128, N], F16, tag="mk")
                    for half in range(2):
                        ps = ps_s.tile([128, 1024], F32, tag="s")
                        for nch in range(2):
                            o = half * 1024 + nch * MC
                            nc.tensor.matmul(
                                ps[:, nch * MC:(nch + 1) * MC],
                                tkh[:, t0 + i * 128:t0 + (i + 1) * 128],
                                tqb[b][:, o:o + MC],
                                start=True, stop=True)
                        nc.scalar.activation(
                            e[:, half * 1024:(half + 1) * 1024], ps[:],
                            mybir.ActivationFunctionType.Exp)
                        nc.scalar.activation(
                            m[:, half * 1024:(half + 1) * 1024], ps[:],
                            mybir.ActivationFunctionType.Sigmoid,
                            bias=biask[:], scale=1e8)
                    if mult_on_dve:
                        nc.vector.tensor_tensor(e[:], e[:], m[:],
                                                op=mybir.AluOpType.mult)
                    else:
                        nc.gpsimd.tensor_tensor(e[:], e[:], m[:],
                                                op=mybir.AluOpType.mult)
                    uts.append(e)

                # ---- C: recursion ----
                vcur = vb.tile([128, NT * 65], F16, tag="vb")
                for j in range(NT):
                    pst = ps_w.tile([128, 65], F32, tag="wps")
                    nc.tensor.matmul(
                        pst[:, 0:64],
                        Tvs[h][0:DH, t0 + j * 128:t0 + (j + 1) * 128],
                        eye[:], start=True, stop=True)
                    nc.scalar.copy(vcur[:, j * 65:j * 65 + 64], pst[:, 0:64])
                nc.vector.memset(
                    vcur[:].rearrange("p (j o) -> p j o", o=65)[:, :, 64:65],
                    1.0)

                pair = h * 2 + b
                vsteps = []
                vc = vcur
                for step in range(ORDER):
                    vnxt = vb.tile([128, NT * 65], F16, tag="vb")
                    for j in range(NT):
                        ps = ps_w.tile([128, 65], F32, tag="wps")
                        for i in range(NT):
                            nc.tensor.matmul(
                                ps[:], uts[i][:, j * 128:(j + 1) * 128],
                                vc[:, i * 65:(i + 1) * 65],
                                start=(i == 0), stop=(i == NT - 1))
                        rd = small.tile([128, 1], F32, tag="rd")
                        nc.vector.reciprocal(rd[:], ps[:, 64:65])
                        nc.vector.tensor_scalar(
                            vnxt[:, j * 65:(j + 1) * 65], ps[:], rd[:], None,
                            op0=mybir.AluOpType.mult)
                    vsteps.append(vnxt)
                    vc = vnxt

                # res^T = sum_step alpha*I applied to v_step, short PSUM tail
                rp = []
                for _q in range(2):
                    rpt = ps_s.tile([64, 1024], F32, tag="s")
                    rp.append(rpt)
                    for bk in range(2):
                        nc.tensor.matmul(rpt[:, bk * MC:(bk + 1) * MC],
                                         eye[:], zeros[:],
                                         start=True, stop=False)
                for step in range(ORDER):
                    for j in range(NT):
                        aslot = pair * ORDER + step
                        nc.tensor.matmul(
                            rp[j // 8][:, (j % 8) * 128:(j % 8 + 1) * 128],
                            vsteps[step][:, j * 65:j * 65 + 64],
                            aIt[:, aslot * 128:(aslot + 1) * 128],
                            start=False, stop=(step == ORDER - 1))

                for q in range(2):
                    nc.scalar.copy(
                        resT[h * DH:(h + 1) * DH,
                             t0 + q * 1024:t0 + (q + 1) * 1024], rp[q][:])

            phase_A(0)
            if h == 0:
                qkv_sweep([2])             # v0|v1 after first peel is rolling
            phase_A(1)
            phase_BC(0)
            phase_BC(1, mult_on_dve=True)

        # ---------------- out projection (partial) ----------------
        for tt in range(TOK // 128):
            for oc in range(DIM // MC):
                ps = ps_o.tile([128, MC], F32, tag="o")
                nc.tensor.matmul(
                    ps[:], resT[:, tt * 128:(tt + 1) * 128],
                    woutt[:, oc * MC:(oc + 1) * MC], start=True, stop=True)
                ot = xpool.tile([128, MC], F32, tag="ot")
                nc.scalar.copy(ot[:], ps[:])
                nc.sync.dma_start(
                    out_d[tt * 128:(tt + 1) * 128, oc * MC:(oc + 1) * MC],
                    ot[:])

    nc.compile()
    return nc


_NC = None
_LAST_IN_MAPS = None


def make_in_maps(x, Wqkv, bqkv, Wout, bout, alphas_raw):
    x = np.asarray(x, np.float32)
    Wqkv = np.asarray(Wqkv, np.float32)
    bqkv = np.asarray(bqkv, np.float32)
    Wout = np.asarray(Wout, np.float32)
    alphas = 1.0 / (1.0 + np.exp(-np.asarray(alphas_raw, np.float32)))

    xt = np.concatenate([x.reshape(TOK, DIM).T,
                         np.ones((1, TOK), np.float32)], 0).astype(np.float32)
    ones = np.ones((1, TOK), np.float32)
    eye128 = np.eye(128, dtype=np.float32)
    eye64 = np.eye(64, dtype=np.float16)

    inner = H * DH
    wb = np.concatenate([Wqkv, bqkv[None, :]], 0)   # [1025, 3072]
    in_maps = []
    for c in range(8):
        hs = [2 * c, 2 * c + 1]
        cols = []
        for h in hs:                       # q_h | k_h per head
            cols.append(np.arange(h * DH, (h + 1) * DH))
            cols.append(np.arange(inner + h * DH, inner + (h + 1) * DH))
        for h in hs:                       # v0 | v1
            cols.append(np.arange(2 * inner + h * DH,
                                  2 * inner + (h + 1) * DH))
        cols = np.concatenate(cols)
        wc = wb[:, cols].copy()
        wc[:, 0:DH] *= SCALE               # q0
        wc[:, 128:128 + DH] *= SCALE       # q1
        rows = np.concatenate([np.arange(h * DH, (h + 1) * DH) for h in hs])
        woc = Wout[rows, :]
        aI = np.zeros((2 * HPC * ORDER, 128, 128), np.float32)
        for pair in range(2 * HPC):
            h = pair // 2                  # pair = h*2 + b
            for step in range(ORDER):
                aI[pair * ORDER + step] = alphas[step + 1, hs[h]] * eye128
        in_maps.append({
            "xt": xt,
            "w": wc.astype(np.float32),
            "wout": woc.astype(np.float16),
            "aI": aI.astype(np.float16),
            "ones": ones,
            "eye64": eye64,
        })
    return in_maps


def kernel(x, Wqkv, bqkv, Wout, bout, alphas_raw):
    global _NC, _LAST_IN_MAPS
    bout = np.asarray(bout, np.float32)
    in_maps = make_in_maps(x, Wqkv, bqkv, Wout, bout, alphas_raw)
    _LAST_IN_MAPS = in_maps
    if _NC is None:
        _NC = build_nc()
    res = run_bass_kernel_spmd(_NC, in_maps, core_ids=list(range(8)))
    out = np.zeros((TOK, DIM), np.float64)
    for c in range(8):
        out += res.results[c]["out"].astype(np.float64)
    out = (out + bout[None, :]).astype(np.float32)
    return out.reshape(B, N, DIM)
